# revision 5
# baseline (speedup 1.0000x reference)
"""LoFTR LocalFeatureTransformer on 8 NeuronCores (Trainium2 via PJRT).

Sharding: data-parallel over the 8 independent sequences (4 batches x
{feat0, feat1}), one sequence per NeuronCore. Every layer applies the same
weights to all 8 sequences, so self-attention layers are fully local.
Cross-attention layers only need the partner sequence's linear-attention
KV statistics ([H,D,D] + [H,D] ~ 33KB), exchanged with a pairwise
ppermute instead of moving the full 4.7MB sequence.

Device i holds: i even -> feat0[i//2], i odd -> feat1[i//2]; partner = i^1.

Compilation notes (neuronx-cc in this container):
- Fusing all 8 layers into one module, or passing weights as pmap
  parameters, trips an internal tensorizer assertion (MaskPropagation /
  "Need to split to perfect loopnest").
- One pmap module per layer with the layer's weights baked in as
  constants compiles cleanly, so that is what we do. Activations stay
  device-resident between the 8 calls.
"""

import functools

import jax
import jax.numpy as jnp
import numpy as np

D_MODEL = 256
NHEAD = 8
HEAD_DIM = D_MODEL // NHEAD
NLAYERS = 8
LAYER_NAMES = ('self', 'cross', 'self', 'cross', 'self', 'cross', 'self', 'cross')
LN_EPS = 1e-5
ATTN_EPS = 1e-6
N_CORES = 8
AXIS = 'c'

_SWAP = [(i, i ^ 1) for i in range(N_CORES)]


def _elu1(x):
    # elu(x) + 1 == max(x, 0) + exp(min(x, 0)); select-free for the tensorizer
    return jnp.maximum(x, 0.0) + jnp.exp(jnp.minimum(x, 0.0))


def _layernorm(x, g, b):
    mu = jnp.mean(x, axis=-1, keepdims=True)
    var = jnp.mean(jnp.square(x - mu), axis=-1, keepdims=True)
    return (x - mu) * jax.lax.rsqrt(var + LN_EPS) * g + b


def _make_layer(Wq, Wk, Wv, Wm, W1, W2, g1, b1, g2, b2, cross):
    Wq, Wk, Wv, Wm = map(jnp.asarray, (Wq, Wk, Wv, Wm))
    W1a = jnp.asarray(W1[:D_MODEL])
    W1b = jnp.asarray(W1[D_MODEL:])
    W2 = jnp.asarray(W2)
    g1, b1, g2, b2 = map(jnp.asarray, (g1, b1, g2, b2))

    def f(x):
        L = x.shape[0]
        q = x @ Wq
        k = x @ Wk
        v = x @ Wv
        Q = _elu1(q).reshape(L, NHEAD, HEAD_DIM).transpose(1, 0, 2)   # [H,L,D]
        K = _elu1(k).reshape(L, NHEAD, HEAD_DIM).transpose(1, 0, 2)
        Vn = (v / L).reshape(L, NHEAD, HEAD_DIM).transpose(1, 0, 2)
        KV = jnp.matmul(K.transpose(0, 2, 1), Vn)                     # [H,D,D]
        Ksum = K.sum(axis=1)                                          # [H,D]
        if cross:
            KV = jax.lax.ppermute(KV, AXIS, _SWAP)
            Ksum = jax.lax.ppermute(Ksum, AXIS, _SWAP)
        den = jnp.matmul(Q, Ksum[:, :, None])                         # [H,L,1]
        Z = 1.0 / (den + ATTN_EPS)
        msg = jnp.matmul(Q, KV) * Z * L                               # [H,L,D]
        msg = msg.transpose(1, 0, 2).reshape(L, D_MODEL)
        msg = _layernorm(msg @ Wm, g1, b1)
        h = jax.nn.relu(x @ W1a + msg @ W1b) @ W2
        h = _layernorm(h, g2, b2)
        return x + h

    return jax.pmap(f, axis_name=AXIS)


_cache = {}


def kernel(feat0, feat1, Wq, Wk, Wv, Wm, W1, W2, g1, b1, g2, b2):
    feat0 = np.asarray(feat0, dtype=np.float32)
    feat1 = np.asarray(feat1, dtype=np.float32)
    N, L, C = feat0.shape

    key = id(Wq)
    if key not in _cache:
        _cache.clear()
        ws = [np.asarray(w, dtype=np.float32)
              for w in (Wq, Wk, Wv, Wm, W1, W2, g1, b1, g2, b2)]
        _cache[key] = [
            _make_layer(*(w[i] for w in ws), cross=(name == 'cross'))
            for i, name in enumerate(LAYER_NAMES)
        ]
    layers = _cache[key]

    seqs = np.empty((N_CORES, L, C), dtype=np.float32)
    seqs[0::2] = feat0
    seqs[1::2] = feat1
    x = jax.device_put_sharded(list(seqs), jax.devices()[:N_CORES])

    for f in layers:
        x = f(x)

    out = np.asarray(x)
    return out[0::2].copy(), out[1::2].copy()


# revision 7
# speedup vs baseline: 88.4984x; 88.4984x over previous
"""LoFTR LocalFeatureTransformer on 8 NeuronCores (Trainium2 via PJRT).

Sharding: data-parallel over the 8 independent sequences (4 batches x
{feat0, feat1}), one sequence per NeuronCore. Every layer applies the same
weights to all 8 sequences, so self-attention layers are fully local.
Cross-attention layers only need the partner sequence's linear-attention
KV statistics ([H,D,D] + [H,D] ~ 33KB), exchanged with a pairwise
ppermute instead of moving the full 4.7MB sequence.

Device i holds: i even -> feat0[i//2], i odd -> feat1[i//2]; partner = i^1.

Compilation notes (neuronx-cc in this container):
- Fusing all 8 layers into one module, or passing weights as pmap
  parameters, trips an internal tensorizer assertion (MaskPropagation /
  "Need to split to perfect loopnest").
- One pmap module per layer with the layer's weights baked in as
  constants compiles cleanly, so that is what we do. Activations stay
  device-resident between the 8 calls.
"""

import functools

import jax
import jax.numpy as jnp
import numpy as np

D_MODEL = 256
NHEAD = 8
HEAD_DIM = D_MODEL // NHEAD
NLAYERS = 8
LAYER_NAMES = ('self', 'cross', 'self', 'cross', 'self', 'cross', 'self', 'cross')
LN_EPS = 1e-5
ATTN_EPS = 1e-6
N_CORES = 8
AXIS = 'c'

_SWAP = [(i, i ^ 1) for i in range(N_CORES)]


def _elu1(x):
    # elu(x) + 1 == max(x, 0) + exp(min(x, 0)); select-free for the tensorizer
    return jnp.maximum(x, 0.0) + jnp.exp(jnp.minimum(x, 0.0))


def _layernorm(x, g, b):
    mu = jnp.mean(x, axis=-1, keepdims=True)
    var = jnp.mean(jnp.square(x - mu), axis=-1, keepdims=True)
    return (x - mu) * jax.lax.rsqrt(var + LN_EPS) * g + b


def _make_layer(Wq, Wk, Wv, Wm, W1, W2, g1, b1, g2, b2, cross):
    Wq, Wk, Wv, Wm = map(jnp.asarray, (Wq, Wk, Wv, Wm))
    W1a = jnp.asarray(W1[:D_MODEL])
    W1b = jnp.asarray(W1[D_MODEL:])
    W2 = jnp.asarray(W2)
    g1, b1, g2, b2 = map(jnp.asarray, (g1, b1, g2, b2))

    def f(x):
        L = x.shape[0]
        q = x @ Wq
        k = x @ Wk
        v = x @ Wv
        Q = _elu1(q).reshape(L, NHEAD, HEAD_DIM).transpose(1, 0, 2)   # [H,L,D]
        K = _elu1(k).reshape(L, NHEAD, HEAD_DIM).transpose(1, 0, 2)
        Vn = (v / L).reshape(L, NHEAD, HEAD_DIM).transpose(1, 0, 2)
        KV = jnp.matmul(K.transpose(0, 2, 1), Vn)                     # [H,D,D]
        Ksum = K.sum(axis=1)                                          # [H,D]
        if cross:
            KV = jax.lax.ppermute(KV, AXIS, _SWAP)
            Ksum = jax.lax.ppermute(Ksum, AXIS, _SWAP)
        den = jnp.matmul(Q, Ksum[:, :, None])                         # [H,L,1]
        Z = 1.0 / (den + ATTN_EPS)
        msg = jnp.matmul(Q, KV) * Z * L                               # [H,L,D]
        msg = msg.transpose(1, 0, 2).reshape(L, D_MODEL)
        msg = _layernorm(msg @ Wm, g1, b1)
        h = jax.nn.relu(x @ W1a + msg @ W1b) @ W2
        h = _layernorm(h, g2, b2)
        return x + h

    return f


def _build_layers(ws):
    """Four pmap modules, each fusing one (self, cross) layer pair.

    2-layer fusion compiles cleanly; full 8-layer fusion trips the
    tensorizer assertion noted above.
    """
    fns = []
    for i in range(0, NLAYERS, 2):
        fa = _make_layer(*(w[i] for w in ws),
                         cross=(LAYER_NAMES[i] == 'cross'))
        fb = _make_layer(*(w[i + 1] for w in ws),
                         cross=(LAYER_NAMES[i + 1] == 'cross'))
        fns.append(jax.pmap(lambda x, fa=fa, fb=fb: fb(fa(x)),
                            axis_name=AXIS))
    return fns


_cache = {}


def kernel(feat0, feat1, Wq, Wk, Wv, Wm, W1, W2, g1, b1, g2, b2):
    feat0 = np.asarray(feat0, dtype=np.float32)
    feat1 = np.asarray(feat1, dtype=np.float32)
    N, L, C = feat0.shape

    key = id(Wq)
    if key not in _cache:
        _cache.clear()
        ws = [np.asarray(w, dtype=np.float32)
              for w in (Wq, Wk, Wv, Wm, W1, W2, g1, b1, g2, b2)]
        _cache[key] = _build_layers(ws)
    layers = _cache[key]

    seqs = np.empty((N_CORES, L, C), dtype=np.float32)
    seqs[0::2] = feat0
    seqs[1::2] = feat1
    x = jax.device_put_sharded(list(seqs), jax.devices()[:N_CORES])

    for f in layers:
        x = f(x)

    out = np.asarray(x)
    return out[0::2].copy(), out[1::2].copy()
